# revision 3
# baseline (speedup 1.0000x reference)
"""HeteroGCN (2-layer, 3-relation GCN, mean-aggregated) on 8 TRN2 NeuronCores.

Strategy: dst-shard nodes across 8 cores. Host buckets each core's incoming
edges (plus self-loops, with symmetric-norm weights folded per edge) by
(relation r, src-chunk k of 25000 rows, dst-block b of 128 nodes), padding
each bucket to whole 128-edge tiles (uniform across cores -> one SPMD
program). Device per tile: dma_gather 128 source rows (bf16, edges on
partitions), DVE builds a w-scaled one-hot [128e x 128n] via fused
tensor_scalar(is_equal, mult), TensorE computes M^T @ P accumulating the
segment sum (transposed agg) in PSUM. Per block: 3 psum->SBUF copies, then
3 fp32 matmuls with W_r (+ bias via ones-row matmul), ReLU -> h shard.
AllGather h across cores between layers; layer 2 repeats with D_OUT=64.
"""

import math
import numpy as np
import ml_dtypes

import concourse.bass as bass
import concourse.tile as tile
import concourse.bass_utils as bass_utils
from concourse import bacc, mybir

BF16 = ml_dtypes.bfloat16

# ---------------------------------------------------------------- config ----
class Cfg:
    def __init__(self, N=100000, E=1600000, R=3, NC=8, D=128, DO=64,
                 CHS=25000, G=3):
        self.N, self.E, self.R, self.NC, self.D, self.DO = N, E, R, NC, D, DO
        self.S = N // NC                      # nodes per core
        self.BLK = 128
        self.NBLK = (self.S + 127) // 128     # dst blocks per core
        self.CHS = CHS                        # src chunk rows (int16 reach)
        self.NCH = (N + CHS - 1) // CHS
        self.G = G                            # blocks per gather window
        self.NW = (self.NBLK + G - 1) // G


def cdiv(a, b):
    return (a + b - 1) // b


# ---------------------------------------------------------- preprocessing ----
def preprocess(cfg, edge_index):
    """Bucket/pad edges. Returns (meta, per-core arrays)."""
    N, R, NC, CHS = cfg.N, cfg.R, cfg.NC, cfg.CHS
    S, BLK, NBLK, NCH = cfg.S, cfg.BLK, cfg.NBLK, cfg.NCH

    loops = np.arange(N, dtype=np.int64)
    srcs, dsts, ws, rs = [], [], [], []
    dinv_all = np.zeros((R, N), np.float64)
    for r in range(R):
        s = np.asarray(edge_index[r, 0], dtype=np.int64)
        d = np.asarray(edge_index[r, 1], dtype=np.int64)
        deg = np.bincount(d, minlength=N).astype(np.float64) + 1.0
        dinv = 1.0 / np.sqrt(deg)
        dinv_all[r] = dinv
        s2 = np.concatenate([s, loops])
        d2 = np.concatenate([d, loops])
        srcs.append(s2); dsts.append(d2)
        ws.append((dinv[s2] * dinv[d2] / R).astype(np.float32))
        rs.append(np.full(s2.shape[0], r, np.int64))
    src = np.concatenate(srcs); dst = np.concatenate(dsts)
    w = np.concatenate(ws); rel = np.concatenate(rs)

    core = dst // S
    dl = dst - core * S
    b = dl // BLK
    slot = dl - b * BLK
    k = src // CHS
    sl = (src - k * CHS).astype(np.int32)
    bucket = ((core * R + rel) * NBLK + b) * NCH + k
    order = np.argsort(bucket * CHS + sl, kind="stable")
    bucket_s = bucket[order]; sl_s = sl[order]
    slot_s = slot[order].astype(np.float32); w_s = w[order]

    nbuck = NC * R * NBLK * NCH
    counts = np.bincount(bucket_s, minlength=nbuck).reshape(NC, R, NBLK, NCH)
    starts = np.concatenate([[0], np.cumsum(counts.reshape(-1))[:-1]]
                            ).reshape(NC, R, NBLK, NCH)
    T = cdiv(counts.max(axis=0), 128)         # [R, NBLK, NCH] tiles per bucket

    # stream tile offsets per (r,k): blocks concatenated
    Boff = np.zeros((R, NCH, NBLK), np.int64)
    Trk = np.zeros((R, NCH), np.int64)
    for r in range(R):
        for kk in range(NCH):
            c = np.cumsum(T[r, :, kk])
            Boff[r, kk, 1:] = c[:-1]
            Trk[r, kk] = c[-1]
    IDXoff = np.zeros((R, NCH), np.int64)     # tile col offset of stream (r,k)
    acc = 0
    for r in range(R):
        for kk in range(NCH):
            IDXoff[r, kk] = acc
            acc += Trk[r, kk]
    Ttot = acc

    IDX = np.zeros((NC, 128, 8 * Ttot), np.int16)
    SLOT = np.full((NC, 128, Ttot), 512.0, np.float32)
    WV = np.zeros((NC, 128, Ttot), np.float32)
    g8 = (16 * np.arange(8))[:, None]
    for c in range(NC):
        for r in range(R):
            for kk in range(NCH):
                base_t = IDXoff[r, kk]
                for bb in range(NBLK):
                    cnt = counts[c, r, bb, kk]
                    if cnt == 0:
                        continue
                    st = starts[c, r, bb, kk]
                    i = 128 * Boff[r, kk, bb] + np.arange(cnt)
                    col = 8 * base_t + i // 16
                    IDX[c, (i % 16)[None, :] + g8, col[None, :]] = \
                        sl_s[st:st + cnt].astype(np.int16)[None, :]
                    tc_ = base_t + i // 128
                    SLOT[c, i % 128, tc_] = slot_s[st:st + cnt]
                    WV[c, i % 128, tc_] = w_s[st:st + cnt]

    meta = dict(T=T, Boff=Boff, Trk=Trk, IDXoff=IDXoff, Ttot=int(Ttot),
                counts=counts)
    return meta, IDX, SLOT, WV, dinv_all


# -------------------------------------------------------------- builder ----
def build(cfg, meta):
    nc = bacc.Bacc("TRN2", target_bir_lowering=False, debug=False,
                   num_devices=cfg.NC)
    dt = mybir.dt
    N, R, S, D, DO = cfg.N, cfg.R, cfg.S, cfg.D, cfg.DO
    NBLK, NCH, CHS, G, NW = cfg.NBLK, cfg.NCH, cfg.CHS, cfg.G, cfg.NW
    T, Boff, Trk, IDXoff, Ttot = (meta["T"], meta["Boff"], meta["Trk"],
                                  meta["IDXoff"], meta["Ttot"])

    xt = nc.dram_tensor("xt", [N, D], dt.bfloat16, kind="ExternalInput")
    idx_d = nc.dram_tensor("idx", [128, 8 * Ttot], dt.int16,
                           kind="ExternalInput")
    slot_d = nc.dram_tensor("slot", [128, Ttot], dt.float32,
                            kind="ExternalInput")
    wv_d = nc.dram_tensor("wv", [128, Ttot], dt.float32, kind="ExternalInput")
    w1_d = nc.dram_tensor("w1", [R * D, D], dt.float32, kind="ExternalInput")
    w2_d = nc.dram_tensor("w2", [R * D, DO], dt.float32, kind="ExternalInput")
    b1_d = nc.dram_tensor("b1m", [1, D], dt.float32, kind="ExternalInput")
    b2_d = nc.dram_tensor("b2m", [1, DO], dt.float32, kind="ExternalInput")
    iota_d = nc.dram_tensor("iota", [128, 128], dt.bfloat16,
                            kind="ExternalInput")
    ones_d = nc.dram_tensor("ones", [1, 128], dt.float32, kind="ExternalInput")
    out_d = nc.dram_tensor("out", [S, DO], dt.float32, kind="ExternalOutput")

    with tile.TileContext(nc) as tc:
        with tc.tile_pool(name="consts", bufs=1) as cp, \
             tc.tile_pool(name="dram", bufs=1, space="DRAM") as dp, \
             tc.tile_pool(name="gath", bufs=2) as gp, \
             tc.tile_pool(name="oh", bufs=6) as ohp, \
             tc.tile_pool(name="agg", bufs=6) as aggp, \
             tc.tile_pool(name="sout", bufs=3) as sop, \
             tc.tile_pool(name="psA", bufs=4, space="PSUM") as psA, \
             tc.tile_pool(name="psB", bufs=2, space="PSUM") as psB:

            iota_sb = cp.tile([128, 128], dt.bfloat16)
            nc.sync.dma_start(iota_sb[:], iota_d[:, :])
            w1_sb = cp.tile([128, R * D], dt.float32)
            w2_sb = cp.tile([128, R * DO], dt.float32)
            for r in range(R):
                nc.sync.dma_start(w1_sb[:, r * D:(r + 1) * D],
                                  w1_d[r * D:(r + 1) * D, :])
                nc.sync.dma_start(w2_sb[:, r * DO:(r + 1) * DO],
                                  w2_d[r * D:(r + 1) * D, :])
            ones_sb = cp.tile([1, 128], dt.float32)
            nc.sync.dma_start(ones_sb[:], ones_d[:, :])
            b1_sb = cp.tile([1, D], dt.float32)
            nc.sync.dma_start(b1_sb[:], b1_d[:, :])
            b2_sb = cp.tile([1, DO], dt.float32)
            nc.sync.dma_start(b2_sb[:], b2_d[:, :])

            h_shard = dp.tile([S, D], dt.bfloat16)
            h_full = dp.tile([N, D], dt.bfloat16)

            for layer in range(2):
                fo = D if layer == 0 else DO
                wsb = w1_sb if layer == 0 else w2_sb
                bsb = b1_sb if layer == 0 else b2_sb
                for wdx in range(NW):
                    blo = wdx * G
                    bhi = min(blo + G, NBLK)
                    gt = {}
                    st = {}
                    wt = {}
                    woff = {}
                    for r in range(R):
                        for k in range(NCH):
                            t0 = Boff[r, k, blo]
                            t1 = (Trk[r, k] if bhi == NBLK
                                  else Boff[r, k, bhi])
                            tw = int(t1 - t0)
                            woff[(r, k)] = t0
                            if tw == 0:
                                continue
                            c0 = int(IDXoff[r, k] + t0)
                            gi = gp.tile([128, 8 * tw], dt.int16,
                                         tag=f"i{r}_{k}", name=f"gi{r}{k}")
                            nc.sync.dma_start(
                                gi[:], idx_d[:, 8 * c0:8 * (c0 + tw)])
                            ss = gp.tile([128, tw], dt.float32,
                                         tag=f"s{r}_{k}", name=f"ss{r}{k}")
                            nc.sync.dma_start(ss[:], slot_d[:, c0:c0 + tw])
                            sw = gp.tile([128, tw], dt.float32,
                                         tag=f"w{r}_{k}", name=f"sw{r}{k}")
                            nc.sync.dma_start(sw[:], wv_d[:, c0:c0 + tw])
                            g = gp.tile([128, tw, 128], dt.bfloat16,
                                        tag=f"g{r}_{k}", name=f"g{r}{k}")
                            if layer == 0:
                                tab = xt[k * CHS:min((k + 1) * CHS, N), :]
                            else:
                                tab = h_full[k * CHS:min((k + 1) * CHS, N), :]
                            nc.gpsimd.dma_gather(
                                g[:], tab, gi[:], 128 * tw, 128 * tw, 128,
                                single_packet=False)
                            gt[(r, k)] = g
                            st[(r, k)] = ss
                            wt[(r, k)] = sw
                    for bb in range(blo, bhi):
                        nb = min(cfg.S - bb * 128, 128)
                        aggs = []
                        for r in range(R):
                            ps = psA.tile([128, 128], dt.float32, tag="psA",
                                          name="ps")
                            nmm = int(sum(T[r, bb, k] for k in range(NCH)))
                            mi = 0
                            for k in range(NCH):
                                for t in range(int(T[r, bb, k])):
                                    lt = int(Boff[r, k, bb] - woff[(r, k)] + t)
                                    oh = ohp.tile([128, 128], dt.bfloat16,
                                                  tag="oh", name="oh")
                                    nc.vector.tensor_scalar(
                                        oh[:], iota_sb[:],
                                        st[(r, k)][:, lt:lt + 1],
                                        wt[(r, k)][:, lt:lt + 1],
                                        mybir.AluOpType.is_equal,
                                        mybir.AluOpType.mult)
                                    nc.tensor.matmul(
                                        ps[:], gt[(r, k)][:, lt, :], oh[:],
                                        start=(mi == 0), stop=(mi == nmm - 1))
                                    mi += 1
                            assert mi == nmm and nmm > 0
                            ag = aggp.tile([128, 128], dt.float32, tag="agg",
                                           name="ag")
                            nc.scalar.copy(ag[:], ps[:])
                            aggs.append(ag)
                        op = psB.tile([128, fo], dt.float32, tag="psB",
                                      name="op")
                        for r in range(R):
                            nc.tensor.matmul(op[:], aggs[r][:],
                                             wsb[:, r * fo:(r + 1) * fo],
                                             start=(r == 0), stop=False)
                        nc.tensor.matmul(op[:], ones_sb[:], bsb[:],
                                         start=False, stop=True)
                        if layer == 0:
                            ht = sop.tile([128, D], dt.bfloat16, tag="ht",
                                          name="ht")
                            nc.scalar.activation(
                                ht[:], op[:], mybir.ActivationFunctionType.Relu)
                            nc.sync.dma_start(
                                h_shard[bb * 128:bb * 128 + nb, :], ht[:nb, :])
                        else:
                            ot = sop.tile([128, DO], dt.float32, tag="ot",
                                          name="ot")
                            nc.scalar.copy(ot[:], op[:])
                            nc.sync.dma_start(
                                out_d[bb * 128:bb * 128 + nb, :], ot[:nb, :])
                if layer == 0:
                    nc.gpsimd.collective_compute(
                        "AllGather", mybir.AluOpType.bypass,
                        replica_groups=[list(range(cfg.NC))],
                        ins=[h_shard.opt()], outs=[h_full.opt()])
    nc.compile()
    return nc


# --------------------------------------------------------------- kernel ----
LAST_EXEC_NS = None


def kernel(x, edge_index, W1, b1, W2, b2, cfg=None, trace=False,
           core_ids=None):
    global LAST_EXEC_NS
    cfg = cfg or Cfg()
    x = np.asarray(x, np.float32)
    edge_index = np.asarray(edge_index)
    W1 = np.asarray(W1, np.float32); b1 = np.asarray(b1, np.float32)
    W2 = np.asarray(W2, np.float32); b2 = np.asarray(b2, np.float32)

    meta, IDX, SLOT, WV, dinv = preprocess(cfg, edge_index)
    nc = build(cfg, meta)

    x_bf = x.astype(BF16)
    iota = np.tile(np.arange(128, dtype=np.float32), (128, 1)).astype(BF16)
    w1 = W1.reshape(cfg.R * cfg.D, cfg.D)
    w2 = W2.reshape(cfg.R * cfg.D, cfg.DO)
    b1m = (b1.mean(axis=0) / 1.0).reshape(1, -1).astype(np.float32)
    b2m = (b2.mean(axis=0) / 1.0).reshape(1, -1).astype(np.float32)
    # bias enters as mean over relations (HeteroConv mean aggr):
    b1m = (b1.sum(axis=0) / cfg.R).reshape(1, -1).astype(np.float32)
    b2m = (b2.sum(axis=0) / cfg.R).reshape(1, -1).astype(np.float32)
    ones = np.ones((1, 128), np.float32)

    in_maps = []
    for c in range(cfg.NC):
        in_maps.append(dict(
            xt=x_bf, idx=IDX[c], slot=SLOT[c], wv=WV[c], w1=w1, w2=w2,
            b1m=b1m, b2m=b2m, iota=iota, ones=ones))
    res = bass_utils.run_bass_kernel_spmd(
        nc, in_maps, core_ids=core_ids or list(range(cfg.NC)), trace=trace)
    LAST_EXEC_NS = res.exec_time_ns
    out = np.concatenate([res.results[c]["out"] for c in range(cfg.NC)],
                         axis=0)
    return out.astype(np.float32)
